# revision 16
# baseline (speedup 1.0000x reference)
"""Trainium2 Bass kernel for nn_DeepLatent chamfer+BCE loss.

loss = mean_b [ chamfer(est_b, gt_b) + bce(labels_b, labels_est_b) ]

Strategy: pure data parallel over B=32 across 8 cores (4 batches/core).
Per batch, d2[n,m] = |e_n|^2 + |g_m|^2 - 2 e_n.g_m is produced by the PE
via a K=13 split-fp8(e4m3) contraction (hi+lo pairs give ~2^-8 relative
operand accuracy; products accumulate exactly in fp32 PSUM). The PE is
the critical engine: at the unramped 1.2 GHz p-state the 64x512-column
matmuls per batch are a hard ~27 us/batch floor, so PSUM tiles are
drained in 1024-wide halves interleaved with the matmuls (half 0 drains
while matmuls 3-4 still fill half 1) to keep the PE gapless.

Reductions use the exp-domain (softmin) trick so the mandatory 1x
PSUM->SBUF drain pass does double duty, split across engines:
  - s-tiles: ScalarE activation(Exp, scale=-1/T, accum_out) drains the
    PSUM halves to E=exp(-d2/T) bf16 AND row-sums them (dist1 softmin).
    accE accumulates over s-tiles in two independent chains (VectorE
    and GpSimd) merged on the host.
  - v-tiles: VectorE tensor_copy casts the PSUM halves to bf16, DMA'd
    out raw; the host computes exact mins for both directions.
dist2 on the HOST: min(-T ln(colsum accE), v-tile column mins).
Softmin bias + fp8 noise at T=1/16 is ~-3e-3 relative on the total
loss (BCE dominates), far inside the 2e-2 gate.

BCE: vector parts (relu-sum, t*z, |z| via scalar_tensor_tensor) plus
the Exp pass run at program START hidden under input DMAs sharing the
drains' Exp table; only the Ln+accum pass trails at the end (one table
load, overlapped with the final DMAs).
"""

import os
import numpy as np

B, N = 32, 2048
NCORES = 8
BPC = B // NCORES  # batches per core
NTILES = N // 128  # 16 est tiles per batch

# est tiles drained by VectorE (raw d2 -> host exact min); rest ScalarE exp.
V_TILES = (1, 3, 5, 7, 9, 11, 13)
NV = len(V_TILES)
S_TILES = tuple(i for i in range(NTILES) if i not in V_TILES)
# s-tiles whose accE chain runs on GpSimd (early tiles: the 4 us gpsimd
# ops then finish well before the batch tail), rest chain on VectorE
G_CHAIN = (0, 2, 4, 6, 8, 10)
INV_T = 16.0          # 1/T; T=0.0625 keeps exp(-d2min/T) >= ~1e-31 (bf16 ok)
SLOTC = 24            # per-batch stats: 18 half slots + 3 bce slots

_cache = {}


def _build_program():
    import sys
    if "/opt/trn_rl_repo" not in sys.path:
        sys.path.insert(0, "/opt/trn_rl_repo")
    import concourse.bass as bass
    import concourse.tile as tile
    from concourse import bacc, mybir

    BF16 = mybir.dt.bfloat16
    FP8 = mybir.dt.float8e4
    FP32 = mybir.dt.float32
    AOP = mybir.AluOpType
    AFT = mybir.ActivationFunctionType

    nc = bacc.Bacc("TRN2", target_bir_lowering=False, debug=False)

    estP_d = nc.dram_tensor("estP", [128, BPC * 512], FP8, kind="ExternalInput")
    gtP_d = nc.dram_tensor("gtP", [128, BPC * 2048], FP8, kind="ExternalInput")
    z_d = nc.dram_tensor("zt", [128, BPC * 16], FP32, kind="ExternalInput")
    t_d = nc.dram_tensor("tt", [128, BPC * 16], FP32, kind="ExternalInput")
    accEv_d = nc.dram_tensor("accEv", [128, BPC * 2048], BF16, kind="ExternalOutput")
    accEg_d = nc.dram_tensor("accEg", [128, BPC * 2048], BF16, kind="ExternalOutput")
    vraw_d = nc.dram_tensor("vraw", [128, BPC * NV * 2048], FP8,
                            kind="ExternalOutput")
    slots_d = nc.dram_tensor("slots", [128, SLOTC * BPC], FP32, kind="ExternalOutput")

    with tile.TileContext(nc) as tc:
        with (
            tc.tile_pool(name="const", bufs=1) as cpool,
            tc.tile_pool(name="work", bufs=6) as work_pool,
            tc.tile_pool(name="vout", bufs=3) as vout_pool,
            tc.tile_pool(name="accE", bufs=2) as accEv_pool,
            tc.tile_pool(name="accG", bufs=2) as accEg_pool,
            tc.tile_pool(name="bce", bufs=1) as bce_pool,
            tc.tile_pool(name="stats", bufs=1) as stats_pool,
            tc.tile_pool(name="ps", bufs=4, space=bass.MemorySpace.PSUM) as ps_pool,
        ):
            # ---- load inputs (z/t first: bce runs under the est/gt DMA) ----
            est_sb = cpool.tile([128, BPC * 512], FP8, tag="est")
            gt_sb = cpool.tile([128, BPC * 2048], FP8, tag="gt")
            z_sb = cpool.tile([128, BPC * 16], FP32, tag="z")
            t_sb = cpool.tile([128, BPC * 16], FP32, tag="t")

            nc.sync.dma_start(z_sb[:], z_d[:])
            nc.sync.dma_start(t_sb[:], t_d[:])
            nc.sync.dma_start(est_sb[:], estP_d[:])
            for b in range(BPC):
                bs = slice(2048 * b, 2048 * (b + 1))
                nc.sync.dma_start(gt_sb[:, bs], gtP_d[:, bs])

            stats = stats_pool.tile([128, SLOTC * BPC], FP32)
            nc.vector.memset(stats[:], 0.0)

            # ---- BCE front half (vector parts + Exp sharing drain table) --
            # bce_sum = sum relu(z) + sum log1p(exp(-|z|)) - sum t*z
            sp = bce_pool.tile([128, BPC * 16], FP32, tag="sp")
            rj = bce_pool.tile([128, BPC * 16], FP32, tag="rj")
            tzj = bce_pool.tile([128, BPC * 16], FP32, tag="tzj")
            for b in range(BPC):
                zb = z_sb[:, 16 * b:16 * (b + 1)]
                nc.vector.tensor_scalar(
                    out=rj[:, 16 * b:16 * (b + 1)], in0=zb,
                    scalar1=0.0, scalar2=None,
                    op0=AOP.max, op1=AOP.add,
                    accum_out=stats[:, SLOTC * b + 18:SLOTC * b + 19],
                )
                nc.vector.scalar_tensor_tensor(
                    out=tzj[:, 16 * b:16 * (b + 1)], in0=zb, scalar=-1.0,
                    in1=t_sb[:, 16 * b:16 * (b + 1)],
                    op0=AOP.mult, op1=AOP.mult,
                    accum_out=stats[:, SLOTC * b + 20:SLOTC * b + 21],
                )
                # |z| = max(-z, z) on VectorE (no Abs table needed)
                nc.vector.scalar_tensor_tensor(
                    out=sp[:, 16 * b:16 * (b + 1)], in0=zb, scalar=-1.0,
                    in1=zb, op0=AOP.mult, op1=AOP.max,
                )
            for b in range(BPC):
                nc.scalar.activation(
                    sp[:, 16 * b:16 * (b + 1)], sp[:, 16 * b:16 * (b + 1)],
                    AFT.Exp, scale=-1.0)

            # tiny PE ops that absorb each PE-feeding DMA-completion wait
            # (walrus allows only ONE sync wait on a matmul)
            warm = ps_pool.tile([128, 1024], FP32, tag="ps")
            nc.tensor.matmul(
                warm[0:1, 0:1], est_sb[0:32, 0:1], est_sb[0:32, 0:1],
                start=True, stop=True,
            )
            for b in range(BPC):
                nc.tensor.matmul(
                    warm[0:1, b + 1:b + 2],
                    gt_sb[0:32, 2048 * b:2048 * b + 1],
                    gt_sb[0:32, 2048 * b:2048 * b + 1],
                    start=True, stop=True,
                )

            for b in range(BPC):
                accEv = accEv_pool.tile([128, 2048], BF16)
                accEg = accEg_pool.tile([128, 2048], BF16)
                vchain = []   # pending E tiles for the vector chain
                gchain = []
                vstarted = False
                gstarted = False
                vslot = 0
                for i in range(NTILES):
                    a, c = i % 4, i // 4
                    lhsT = est_sb[32 * a:32 * a + 32,
                                  b * 512 + 128 * c: b * 512 + 128 * (c + 1)]
                    is_v = i in V_TILES
                    if is_v:
                        dsb = vout_pool.tile([128, 2048], FP8, tag="vsb")
                    else:
                        dsb = work_pool.tile([128, 2048], BF16, tag="esb")
                        si = S_TILES.index(i)
                    # each half gets its OWN psum tile: no WAR between the
                    # half-drain and the next pair of matmuls
                    for h in range(2):
                        ps = ps_pool.tile([128, 1024], FP32, tag="ps")
                        for jj in (0, 1):
                            m0 = b * 2048 + 1024 * h + jj * 512
                            nc.tensor.matmul(
                                ps[:, jj * 512:(jj + 1) * 512],
                                lhsT,
                                gt_sb[32 * a:32 * a + 32, m0:m0 + 512],
                                start=True,
                                stop=True,
                                tile_position=(32 * a, 0),
                            )
                        hs = slice(1024 * h, 1024 * (h + 1))
                        if is_v:
                            nc.vector.tensor_copy(dsb[:, hs], ps[:])
                        else:
                            nc.scalar.activation(
                                dsb[:, hs], ps[:], AFT.Exp, scale=-INV_T,
                                accum_out=stats[:, SLOTC * b + 2 * si + h:
                                                SLOTC * b + 2 * si + h + 1],
                            )
                    if is_v:
                        off = (b * NV + vslot) * 2048
                        nc.sync.dma_start(vraw_d[:, off:off + 2048], dsb[:])
                        vslot += 1
                    elif i in G_CHAIN:
                        if gstarted:
                            nc.gpsimd.tensor_tensor(
                                accEg[:], dsb[:], accEg[:], op=AOP.add)
                        elif gchain:
                            nc.gpsimd.tensor_tensor(
                                accEg[:], dsb[:], gchain.pop()[:], op=AOP.add)
                            gstarted = True
                        else:
                            gchain.append(dsb)
                    else:
                        if vstarted:
                            nc.vector.tensor_tensor(
                                accEv[:], dsb[:], accEv[:], op=AOP.add)
                        elif vchain:
                            nc.vector.tensor_tensor(
                                accEv[:], dsb[:], vchain.pop()[:], op=AOP.add)
                            vstarted = True
                        else:
                            vchain.append(dsb)

                nc.sync.dma_start(accEv_d[:, b * 2048:(b + 1) * 2048], accEv[:])
                nc.sync.dma_start(accEg_d[:, b * 2048:(b + 1) * 2048], accEg[:])

            # ---- BCE tail: Ln + accum (one table load, under final DMAs) --
            for b in range(BPC):
                nc.scalar.activation(
                    sp[:, 16 * b:16 * (b + 1)], sp[:, 16 * b:16 * (b + 1)],
                    AFT.Ln, bias=1.0,
                    accum_out=stats[:, SLOTC * b + 19:SLOTC * b + 20],
                )

            nc.sync.dma_start(slots_d[:], stats[:])

    nc.compile()
    return nc


def _pack_inputs(obs_est, obs_gt, labels_est, labels):
    """Build per-core input maps (host-side layout prep only)."""
    import sys
    if "/opt/trn_rl_repo" not in sys.path:
        sys.path.insert(0, "/opt/trn_rl_repo")
    from concourse import mybir

    obs_est = np.ascontiguousarray(obs_est, dtype=np.float32)
    obs_gt = np.ascontiguousarray(obs_gt, dtype=np.float32)
    labels_est = np.ascontiguousarray(labels_est, dtype=np.float32)
    labels = np.ascontiguousarray(labels, dtype=np.float32)

    F8 = mybir.dt.np(mybir.dt.float8e4)

    def split(v):
        hi = v.astype(F8).astype(np.float32)
        lo = (v - hi).astype(F8).astype(np.float32)
        return hi, lo

    # split-precision fp8 operands: d2 = x2 + y2 - 2 e.g with
    #   x2,y2 as e4m3 hi+lo pairs (~2^-8 relative)
    #   e.g  as ehi*ghi + ehi*glo + elo*ghi (products exact in fp32 PSUM)
    x2 = (obs_est ** 2).sum(-1)  # [B, N]
    y2 = (obs_gt ** 2).sum(-1)
    one = np.ones_like(x2)
    x2h, x2l = split(x2)
    y2h, y2l = split(y2)
    eh, el = split(obs_est)  # [B, N, 3]
    gh, gl = split(obs_gt)
    NK = 13
    est13 = np.stack(
        [x2h, x2l, one, one,
         -2 * eh[..., 0], -2 * eh[..., 1], -2 * eh[..., 2],
         -2 * eh[..., 0], -2 * eh[..., 1], -2 * eh[..., 2],
         -2 * el[..., 0], -2 * el[..., 1], -2 * el[..., 2]], axis=1
    )  # [B, 13, N]
    gt13 = np.stack(
        [one, one, y2h, y2l,
         gh[..., 0], gh[..., 1], gh[..., 2],
         gl[..., 0], gl[..., 1], gl[..., 2],
         gh[..., 0], gh[..., 1], gh[..., 2]], axis=1
    )  # [B, 13, N]

    # estP[b, 32a+k, 128c+p] = est13[b, k, (4c+a)*128+p]; rows 13..31 zero
    estP = np.zeros((B, 128, 512), F8)
    est13_t = est13.reshape(B, NK, NTILES, 128)
    for i in range(NTILES):
        a, c = i % 4, i // 4
        estP[:, 32 * a:32 * a + NK, 128 * c:128 * (c + 1)] = est13_t[:, :, i, :]

    # gtP[b, 32a+k, m] = gt13[b, k, m], replicated over the 4 row groups
    gtP = np.zeros((B, 128, 2048), F8)
    for a in range(4):
        gtP[:, 32 * a:32 * a + NK, :] = gt13

    in_maps = []
    for core in range(NCORES):
        bs = slice(core * BPC, (core + 1) * BPC)
        # [BPC,128,X] -> [128, BPC*X] column blocks per batch
        e = estP[bs].transpose(1, 0, 2).reshape(128, BPC * 512)
        g = gtP[bs].transpose(1, 0, 2).reshape(128, BPC * 2048)
        z = labels_est[bs].reshape(BPC, 128, 16).transpose(1, 0, 2).reshape(
            128, BPC * 16)
        t = labels[bs].reshape(BPC, 128, 16).transpose(1, 0, 2).reshape(
            128, BPC * 16)
        in_maps.append({
            "estP": np.ascontiguousarray(e),
            "gtP": np.ascontiguousarray(g),
            "zt": np.ascontiguousarray(z),
            "tt": np.ascontiguousarray(t),
        })
    return in_maps


def _postprocess_core(outmap):
    """Finish one core's reductions on the host -> [BPC, 3] sums:
    [sum relu dist1, sum relu dist2, bce term sum] per batch."""
    T = 1.0 / INV_T
    accE = (np.asarray(outmap["accEv"]).astype(np.float32)
            + np.asarray(outmap["accEg"]).astype(np.float32))
    vraw = np.asarray(outmap["vraw"]).astype(np.float32)   # [128, BPC*NV*2048]
    slots = np.asarray(outmap["slots"]).astype(np.float64)  # [128, SLOTC*BPC]
    out = np.zeros((BPC, 3))
    with np.errstate(divide="ignore"):
        for b in range(BPC):
            sl = slots[:, SLOTC * b:SLOTC * (b + 1)]
            # v-tile raw d2 blocks: [128, NV, 2048]
            vb = vraw[:, b * NV * 2048:(b + 1) * NV * 2048]
            vb = vb.reshape(128, NV, 2048)
            # dist1: softmin rows (s-tiles: two half-sums) + exact (v-tiles)
            ns2 = 2 * len(S_TILES)
            rs = sl[:, 0:ns2:2] + sl[:, 1:ns2:2]     # [128, n_s] row sums of E
            d1s = np.maximum(-T * np.log(rs), 0.0)
            d1v = np.maximum(vb.min(2), 0.0)         # [128, NV]
            # dist2: combine softmin column-sums with exact v-tile mins
            aE = accE[:, 2048 * b:2048 * (b + 1)].astype(np.float64)
            d2col = np.minimum(-T * np.log(aE.sum(0)), vb.min(axis=(0, 1)))
            out[b, 0] = d1s.sum() + d1v.sum()
            out[b, 1] = np.maximum(d2col, 0.0).sum()
            # bce: sum relu(z) + sum log1p(exp(-|z|)) - sum t*z
            out[b, 2] = sl[:, 18].sum() + sl[:, 19].sum() + sl[:, 20].sum()
    return out


def kernel(obs_est, obs_gt, labels_est, labels):
    import sys
    if "/opt/trn_rl_repo" not in sys.path:
        sys.path.insert(0, "/opt/trn_rl_repo")
    from concourse import bass_utils

    if "nc" not in _cache:
        _cache["nc"] = _build_program()
    nc = _cache["nc"]

    in_maps = _pack_inputs(obs_est, obs_gt, labels_est, labels)

    trace = bool(int(os.environ.get("CHAMFER_TRACE", "0")))
    res = bass_utils.run_bass_kernel_spmd(
        nc, in_maps, core_ids=list(range(NCORES)), trace=trace
    )
    _cache["last_result"] = res

    sums = np.stack(
        [_postprocess_core(res.results[c]) for c in range(NCORES)]
    )  # [NCORES, BPC, 3]
    per_sample = sums.sum(-1) / float(N)
    return np.float32(per_sample.mean())


# revision 18
# speedup vs baseline: 1.0133x; 1.0133x over previous
"""Trainium2 Bass kernel for nn_DeepLatent chamfer+BCE loss.

loss = mean_b [ chamfer(est_b, gt_b) + bce(labels_b, labels_est_b) ]

Strategy: pure data parallel over B=32 across 8 cores (4 batches/core).
Per batch, d2[n,m] = |e_n|^2 + |g_m|^2 - 2 e_n.g_m is produced by the PE
via a K=13 split-fp8(e4m3) contraction (hi+lo pairs give ~2^-8 relative
operand accuracy; products accumulate exactly in fp32 PSUM). The PE is
the critical engine: at the unramped 1.2 GHz p-state the 64x512-column
matmuls per batch are a hard ~27 us/batch floor, so PSUM tiles are
drained in 1024-wide halves interleaved with the matmuls (half 0 drains
while matmuls 3-4 still fill half 1) to keep the PE gapless.

Reductions use the exp-domain (softmin) trick so the mandatory 1x
PSUM->SBUF drain pass does double duty, split across engines:
  - s-tiles: ScalarE activation(Exp, scale=-1/T, accum_out) drains the
    PSUM halves to E=exp(-d2/T) bf16 AND row-sums them (dist1 softmin).
    accE accumulates over s-tiles in two independent chains (VectorE
    and GpSimd) merged on the host.
  - v-tiles: VectorE tensor_copy casts the PSUM halves to bf16, DMA'd
    out raw; the host computes exact mins for both directions.
dist2 on the HOST: min(-T ln(colsum accE), v-tile column mins).
Softmin bias + fp8 noise at T=1/16 is ~-3e-3 relative on the total
loss (BCE dominates), far inside the 2e-2 gate.

BCE: vector parts (relu-sum, t*z, |z| via scalar_tensor_tensor) plus
the Exp pass run at program START hidden under input DMAs sharing the
drains' Exp table; only the Ln+accum pass trails at the end (one table
load, overlapped with the final DMAs).
"""

import os
import numpy as np

B, N = 32, 2048
NCORES = 8
BPC = B // NCORES  # batches per core
NTILES = N // 128  # 16 est tiles per batch

# est tiles drained by VectorE (raw d2 -> host exact min); rest ScalarE exp.
V_TILES = (3, 5, 7, 9, 11, 13, 15)
NV = len(V_TILES)
S_TILES = tuple(i for i in range(NTILES) if i not in V_TILES)
# s-tiles whose accE chain runs on GpSimd (early tiles: the 4 us gpsimd
# ops then finish well before the batch tail), rest chain on VectorE
G_CHAIN = (0, 1, 2, 4, 6, 8)
INV_T = 16.0          # 1/T; T=0.0625 keeps exp(-d2min/T) >= ~1e-31 (bf16 ok)
SLOTC = 24            # per-batch stats: 18 half slots + 3 bce slots

_cache = {}


def _build_program():
    import sys
    if "/opt/trn_rl_repo" not in sys.path:
        sys.path.insert(0, "/opt/trn_rl_repo")
    import concourse.bass as bass
    import concourse.tile as tile
    from concourse import bacc, mybir

    BF16 = mybir.dt.bfloat16
    FP8 = mybir.dt.float8e4
    FP32 = mybir.dt.float32
    AOP = mybir.AluOpType
    AFT = mybir.ActivationFunctionType

    nc = bacc.Bacc("TRN2", target_bir_lowering=False, debug=False)

    estP_d = nc.dram_tensor("estP", [128, BPC * 512], FP8, kind="ExternalInput")
    gtP_d = nc.dram_tensor("gtP", [128, BPC * 2048], FP8, kind="ExternalInput")
    z_d = nc.dram_tensor("zt", [128, BPC * 16], FP32, kind="ExternalInput")
    t_d = nc.dram_tensor("tt", [128, BPC * 16], FP32, kind="ExternalInput")
    accEv_d = nc.dram_tensor("accEv", [128, BPC * 2048], BF16, kind="ExternalOutput")
    accEg_d = nc.dram_tensor("accEg", [128, BPC * 2048], BF16, kind="ExternalOutput")
    vraw_d = nc.dram_tensor("vraw", [128, BPC * NV * 2048], FP8,
                            kind="ExternalOutput")
    slots_d = nc.dram_tensor("slots", [128, SLOTC * BPC], FP32, kind="ExternalOutput")

    with tile.TileContext(nc) as tc:
        with (
            tc.tile_pool(name="const", bufs=1) as cpool,
            tc.tile_pool(name="work", bufs=10) as work_pool,
            tc.tile_pool(name="vout", bufs=5) as vout_pool,
            tc.tile_pool(name="accE", bufs=2) as accEv_pool,
            tc.tile_pool(name="accG", bufs=2) as accEg_pool,
            tc.tile_pool(name="bce", bufs=1) as bce_pool,
            tc.tile_pool(name="stats", bufs=1) as stats_pool,
            tc.tile_pool(name="ps", bufs=4, space=bass.MemorySpace.PSUM) as ps_pool,
        ):
            # ---- load inputs (z/t first: bce runs under the est/gt DMA) ----
            est_sb = cpool.tile([128, BPC * 512], FP8, tag="est")
            gt_sb = cpool.tile([128, BPC * 2048], FP8, tag="gt")
            z_sb = cpool.tile([128, BPC * 16], FP32, tag="z")
            t_sb = cpool.tile([128, BPC * 16], FP32, tag="t")

            nc.sync.dma_start(z_sb[:], z_d[:])
            nc.sync.dma_start(t_sb[:], t_d[:])
            nc.sync.dma_start(est_sb[:], estP_d[:])
            for b in range(BPC):
                bs = slice(2048 * b, 2048 * (b + 1))
                nc.sync.dma_start(gt_sb[:, bs], gtP_d[:, bs])

            stats = stats_pool.tile([128, SLOTC * BPC], FP32)
            nc.vector.memset(stats[:], 0.0)

            # ---- BCE front half (vector parts + Exp sharing drain table) --
            # bce_sum = sum relu(z) + sum log1p(exp(-|z|)) - sum t*z
            sp = bce_pool.tile([128, BPC * 16], FP32, tag="sp")
            rj = bce_pool.tile([128, BPC * 16], FP32, tag="rj")
            tzj = bce_pool.tile([128, BPC * 16], FP32, tag="tzj")
            for b in range(BPC):
                zb = z_sb[:, 16 * b:16 * (b + 1)]
                nc.vector.tensor_scalar(
                    out=rj[:, 16 * b:16 * (b + 1)], in0=zb,
                    scalar1=0.0, scalar2=None,
                    op0=AOP.max, op1=AOP.add,
                    accum_out=stats[:, SLOTC * b + 18:SLOTC * b + 19],
                )
                nc.vector.scalar_tensor_tensor(
                    out=tzj[:, 16 * b:16 * (b + 1)], in0=zb, scalar=-1.0,
                    in1=t_sb[:, 16 * b:16 * (b + 1)],
                    op0=AOP.mult, op1=AOP.mult,
                    accum_out=stats[:, SLOTC * b + 20:SLOTC * b + 21],
                )
                # |z| = max(-z, z) on VectorE (no Abs table needed)
                nc.vector.scalar_tensor_tensor(
                    out=sp[:, 16 * b:16 * (b + 1)], in0=zb, scalar=-1.0,
                    in1=zb, op0=AOP.mult, op1=AOP.max,
                )
            for b in range(BPC):
                nc.scalar.activation(
                    sp[:, 16 * b:16 * (b + 1)], sp[:, 16 * b:16 * (b + 1)],
                    AFT.Exp, scale=-1.0)

            # tiny PE ops that absorb each PE-feeding DMA-completion wait
            # (walrus allows only ONE sync wait on a matmul)
            warm = ps_pool.tile([128, 1024], FP32, tag="ps")
            nc.tensor.matmul(
                warm[0:1, 0:1], est_sb[0:32, 0:1], est_sb[0:32, 0:1],
                start=True, stop=True,
            )
            for b in range(BPC):
                nc.tensor.matmul(
                    warm[0:1, b + 1:b + 2],
                    gt_sb[0:32, 2048 * b:2048 * b + 1],
                    gt_sb[0:32, 2048 * b:2048 * b + 1],
                    start=True, stop=True,
                )

            for b in range(BPC):
                accEv = accEv_pool.tile([128, 2048], BF16)
                accEg = accEg_pool.tile([128, 2048], BF16)
                vchain = []   # pending E tiles for the vector chain
                gchain = []
                vstarted = False
                gstarted = False
                vslot = 0
                for i in range(NTILES):
                    a, c = i % 4, i // 4
                    lhsT = est_sb[32 * a:32 * a + 32,
                                  b * 512 + 128 * c: b * 512 + 128 * (c + 1)]
                    is_v = i in V_TILES
                    if is_v:
                        dsb = vout_pool.tile([128, 2048], FP8, tag="vsb")
                    else:
                        dsb = work_pool.tile([128, 2048], BF16, tag="esb")
                        si = S_TILES.index(i)
                    # each half gets its OWN psum tile: no WAR between the
                    # half-drain and the next pair of matmuls
                    for h in range(2):
                        ps = ps_pool.tile([128, 1024], FP32, tag="ps")
                        for jj in (0, 1):
                            m0 = b * 2048 + 1024 * h + jj * 512
                            nc.tensor.matmul(
                                ps[:, jj * 512:(jj + 1) * 512],
                                lhsT,
                                gt_sb[32 * a:32 * a + 32, m0:m0 + 512],
                                start=True,
                                stop=True,
                                tile_position=(32 * a, 0),
                            )
                        hs = slice(1024 * h, 1024 * (h + 1))
                        if is_v:
                            nc.vector.tensor_copy(dsb[:, hs], ps[:])
                        else:
                            nc.scalar.activation(
                                dsb[:, hs], ps[:], AFT.Exp, scale=-INV_T,
                                accum_out=stats[:, SLOTC * b + 2 * si + h:
                                                SLOTC * b + 2 * si + h + 1],
                            )
                    if is_v:
                        off = (b * NV + vslot) * 2048
                        nc.sync.dma_start(vraw_d[:, off:off + 2048], dsb[:])
                        vslot += 1
                    elif i in G_CHAIN:
                        if gstarted:
                            nc.gpsimd.tensor_tensor(
                                accEg[:], dsb[:], accEg[:], op=AOP.add)
                        elif gchain:
                            nc.gpsimd.tensor_tensor(
                                accEg[:], dsb[:], gchain.pop()[:], op=AOP.add)
                            gstarted = True
                        else:
                            gchain.append(dsb)
                    else:
                        if vstarted:
                            nc.vector.tensor_tensor(
                                accEv[:], dsb[:], accEv[:], op=AOP.add)
                        elif vchain:
                            nc.vector.tensor_tensor(
                                accEv[:], dsb[:], vchain.pop()[:], op=AOP.add)
                            vstarted = True
                        else:
                            vchain.append(dsb)

                nc.sync.dma_start(accEv_d[:, b * 2048:(b + 1) * 2048], accEv[:])
                nc.sync.dma_start(accEg_d[:, b * 2048:(b + 1) * 2048], accEg[:])

            # ---- BCE tail: Ln + accum (one table load, under final DMAs) --
            for b in range(BPC):
                nc.scalar.activation(
                    sp[:, 16 * b:16 * (b + 1)], sp[:, 16 * b:16 * (b + 1)],
                    AFT.Ln, bias=1.0,
                    accum_out=stats[:, SLOTC * b + 19:SLOTC * b + 20],
                )

            nc.sync.dma_start(slots_d[:], stats[:])

    nc.compile()
    return nc


def _pack_inputs(obs_est, obs_gt, labels_est, labels):
    """Build per-core input maps (host-side layout prep only)."""
    import sys
    if "/opt/trn_rl_repo" not in sys.path:
        sys.path.insert(0, "/opt/trn_rl_repo")
    from concourse import mybir

    obs_est = np.ascontiguousarray(obs_est, dtype=np.float32)
    obs_gt = np.ascontiguousarray(obs_gt, dtype=np.float32)
    labels_est = np.ascontiguousarray(labels_est, dtype=np.float32)
    labels = np.ascontiguousarray(labels, dtype=np.float32)

    F8 = mybir.dt.np(mybir.dt.float8e4)

    def split(v):
        hi = v.astype(F8).astype(np.float32)
        lo = (v - hi).astype(F8).astype(np.float32)
        return hi, lo

    # split-precision fp8 operands: d2 = x2 + y2 - 2 e.g with
    #   x2,y2 as e4m3 hi+lo pairs (~2^-8 relative)
    #   e.g  as ehi*ghi + ehi*glo + elo*ghi (products exact in fp32 PSUM)
    x2 = (obs_est ** 2).sum(-1)  # [B, N]
    y2 = (obs_gt ** 2).sum(-1)
    one = np.ones_like(x2)
    x2h, x2l = split(x2)
    y2h, y2l = split(y2)
    eh, el = split(obs_est)  # [B, N, 3]
    gh, gl = split(obs_gt)
    NK = 13
    est13 = np.stack(
        [x2h, x2l, one, one,
         -2 * eh[..., 0], -2 * eh[..., 1], -2 * eh[..., 2],
         -2 * eh[..., 0], -2 * eh[..., 1], -2 * eh[..., 2],
         -2 * el[..., 0], -2 * el[..., 1], -2 * el[..., 2]], axis=1
    )  # [B, 13, N]
    gt13 = np.stack(
        [one, one, y2h, y2l,
         gh[..., 0], gh[..., 1], gh[..., 2],
         gl[..., 0], gl[..., 1], gl[..., 2],
         gh[..., 0], gh[..., 1], gh[..., 2]], axis=1
    )  # [B, 13, N]

    # estP[b, 32a+k, 128c+p] = est13[b, k, (4c+a)*128+p]; rows 13..31 zero
    estP = np.zeros((B, 128, 512), F8)
    est13_t = est13.reshape(B, NK, NTILES, 128)
    for i in range(NTILES):
        a, c = i % 4, i // 4
        estP[:, 32 * a:32 * a + NK, 128 * c:128 * (c + 1)] = est13_t[:, :, i, :]

    # gtP[b, 32a+k, m] = gt13[b, k, m], replicated over the 4 row groups
    gtP = np.zeros((B, 128, 2048), F8)
    for a in range(4):
        gtP[:, 32 * a:32 * a + NK, :] = gt13

    in_maps = []
    for core in range(NCORES):
        bs = slice(core * BPC, (core + 1) * BPC)
        # [BPC,128,X] -> [128, BPC*X] column blocks per batch
        e = estP[bs].transpose(1, 0, 2).reshape(128, BPC * 512)
        g = gtP[bs].transpose(1, 0, 2).reshape(128, BPC * 2048)
        z = labels_est[bs].reshape(BPC, 128, 16).transpose(1, 0, 2).reshape(
            128, BPC * 16)
        t = labels[bs].reshape(BPC, 128, 16).transpose(1, 0, 2).reshape(
            128, BPC * 16)
        in_maps.append({
            "estP": np.ascontiguousarray(e),
            "gtP": np.ascontiguousarray(g),
            "zt": np.ascontiguousarray(z),
            "tt": np.ascontiguousarray(t),
        })
    return in_maps


def _postprocess_core(outmap):
    """Finish one core's reductions on the host -> [BPC, 3] sums:
    [sum relu dist1, sum relu dist2, bce term sum] per batch."""
    T = 1.0 / INV_T
    accE = (np.asarray(outmap["accEv"]).astype(np.float32)
            + np.asarray(outmap["accEg"]).astype(np.float32))
    vraw = np.asarray(outmap["vraw"]).astype(np.float32)   # [128, BPC*NV*2048]
    slots = np.asarray(outmap["slots"]).astype(np.float64)  # [128, SLOTC*BPC]
    out = np.zeros((BPC, 3))
    with np.errstate(divide="ignore"):
        for b in range(BPC):
            sl = slots[:, SLOTC * b:SLOTC * (b + 1)]
            # v-tile raw d2 blocks: [128, NV, 2048]
            vb = vraw[:, b * NV * 2048:(b + 1) * NV * 2048]
            vb = vb.reshape(128, NV, 2048)
            # dist1: softmin rows (s-tiles: two half-sums) + exact (v-tiles)
            ns2 = 2 * len(S_TILES)
            rs = sl[:, 0:ns2:2] + sl[:, 1:ns2:2]     # [128, n_s] row sums of E
            d1s = np.maximum(-T * np.log(rs), 0.0)
            d1v = np.maximum(vb.min(2), 0.0)         # [128, NV]
            # dist2: combine softmin column-sums with exact v-tile mins
            aE = accE[:, 2048 * b:2048 * (b + 1)].astype(np.float64)
            d2col = np.minimum(-T * np.log(aE.sum(0)), vb.min(axis=(0, 1)))
            out[b, 0] = d1s.sum() + d1v.sum()
            out[b, 1] = np.maximum(d2col, 0.0).sum()
            # bce: sum relu(z) + sum log1p(exp(-|z|)) - sum t*z
            out[b, 2] = sl[:, 18].sum() + sl[:, 19].sum() + sl[:, 20].sum()
    return out


def kernel(obs_est, obs_gt, labels_est, labels):
    import sys
    if "/opt/trn_rl_repo" not in sys.path:
        sys.path.insert(0, "/opt/trn_rl_repo")
    from concourse import bass_utils

    if "nc" not in _cache:
        _cache["nc"] = _build_program()
    nc = _cache["nc"]

    in_maps = _pack_inputs(obs_est, obs_gt, labels_est, labels)

    trace = bool(int(os.environ.get("CHAMFER_TRACE", "0")))
    res = bass_utils.run_bass_kernel_spmd(
        nc, in_maps, core_ids=list(range(NCORES)), trace=trace
    )
    _cache["last_result"] = res

    sums = np.stack(
        [_postprocess_core(res.results[c]) for c in range(NCORES)]
    )  # [NCORES, BPC, 3]
    per_sample = sums.sum(-1) / float(N)
    return np.float32(per_sample.mean())
